# revision 47
# baseline (speedup 1.0000x reference)
"""Multi-head attention with ALiBi bias, causal — TRN2 Bass kernel, 8-core SPMD.

Problem: x[2,2048,1024] -> QKV proj (H=16 heads, dh=64) -> per-head causal
attention with ALiBi bias slope_h*(i-j) -> out proj Wo + bo.

Sharding: 2 heads per core (head/tensor parallel). Each core reads full x
(bf16), its slice of Wq/Wk/Wv/Wo (bf16, pre-tiled per-partition-major), and
writes a bf16 partial of the full output; host sums the 8 partials (+bo).

v2 layout (all matmul operands bf16; cost model charges 1 cycle/row at any
moving size for bf16, so skinny matmuls are cheap):
  - qT/kT transposed activations [dh x 2 heads, B*N] via W-stationary proj.
  - v computed DIRECTLY in natural layout [row, dh] via x-tile-stationary
    proj (N=128 moving) -- no PE transpose pass, no staging copy.
  - scores^T tiles [j 128, i<=512] per head; ALiBi folds into softmax as
    exp(s - slope*j_loc) with the per-tile constant c_jt = exp(-128*slope*jt)
    on the V blocks (+ their ones-column, which makes attn@v also emit the
    softmax denominator l for free).
  - attn@v runs OUTPUT-NATURAL: lhsT = pt (scores^T block [j,128i]),
    rhs = vks [j, 65] -> po2 [i, 65] accumulated over j-tiles. 65-wide
    moving operand => ~2x cheaper than streaming i. Diagonal j-tiles skip
    their fully-masked i-blocks entirely; the in-tile triangle is a [128,128]
    bf16 mask on pt.
  - normalize per chunk: reciprocal of l column + one broadcast
    tensor_tensor per head -> A_nat bf16; one [128,128] PE transpose per
    q-tile rebuilds aT for the Wo stage.
  - Wo: aT-stationary, N=512 halves; partial out staged bf16, summed on
    host in float64.
  - slot 1 (steep-slope heads 0-7) keeps only the first 4 j-tiles: ALiBi
    weight of key j relative to j=0 is exp(-slope*j); beyond 512 keys the
    relative weight is < e^-28 (far below bf16 noise).
"""

import numpy as np

import concourse.bass as bass
from concourse import bacc
import concourse.mybir as mybir
from concourse.bass_utils import run_bass_kernel_spmd
from concourse.masks import make_identity, make_upper_triangular
from concourse.tile import TileContext

B, N, D, H, DH = 2, 2048, 1024, 16, 64
NCORES = 8
HPC = H // NCORES          # heads per core = 2
NB = B * N                 # 4096 flattened rows
KT = D // 128              # 8 contraction tiles for the projections
JT_PER_B = N // 128        # 16 j-tiles per batch
CC_PER_B = N // 512        # 4 q-chunks of 512 per batch
NCHUNK = B * CC_PER_B
# Core c owns global heads (15-c, c). Slot 1 (heads 0-7, steepest slope
# h7: 128*s=8) needs only 4 j-tiles; slot 0 (heads 8-15) keeps all 16.
JT_CAPS = (JT_PER_B, 4)

f32 = mybir.dt.float32
bf16 = mybir.dt.bfloat16

AF = mybir.ActivationFunctionType
ALU = mybir.AluOpType


def build_program(repeat=1):
    nc = bacc.Bacc("TRN2", target_bir_lowering=False, debug=False,
                   num_devices=NCORES)

    xT = nc.dram_tensor("xT", [D, NB], bf16, kind="ExternalInput").ap()
    # weights pre-tiled on host to [128, kt*128] so loads are contiguous
    wq = nc.dram_tensor("wq", [128, KT * 128], bf16, kind="ExternalInput").ap()
    wk = nc.dram_tensor("wk", [128, KT * 128], bf16, kind="ExternalInput").ap()
    wv = nc.dram_tensor("wv", [128, KT * 128], bf16, kind="ExternalInput").ap()
    wo = nc.dram_tensor("wo", [HPC * DH, D], bf16, kind="ExternalInput").ap()
    # packed per-partition constants: cv (CC*4*HPC) | vcol (B*JT*HPC) |
    # jbias (HPC) -- one DMA instead of four
    NCONST = CC_PER_B * 4 * HPC + B * JT_PER_B * HPC + HPC
    consts = nc.dram_tensor("consts", [128, NCONST], bf16,
                            kind="ExternalInput").ap()
    out = nc.dram_tensor("out", [NB, D], bf16, kind="ExternalOutput").ap()

    with TileContext(nc) as tc:
        with (
            tc.tile_pool(name="const", bufs=1) as cpool,
            tc.tile_pool(name="persist", bufs=1) as wpool,
            tc.tile_pool(name="xtp", bufs=2) as xtpool,
            tc.tile_pool(name="pt", bufs=3) as ptpool,
            tc.tile_pool(name="small", bufs=2) as spool,
            tc.tile_pool(name="outs", bufs=2) as opool,
            tc.tile_pool(name="ps", bufs=1, space="PSUM") as pspool,
        ):
            # ---- weights on the sync (SP) HWDGE queue, interleaved with the
            # first x chunk so the first q matmul can start ASAP ----
            wqs = cpool.tile([128, KT, 128], bf16, name="wqs")
            nc.sync.dma_start(out=wqs, in_=wq.rearrange("p (t m) -> p t m",
                                                        t=KT))
            xt0 = [xtpool.tile([128, KT // 2, 512], bf16, tag=f"x{h}",
                               bufs=3, name=f"xtc_0{h}") for h in range(2)]
            nc.sync.dma_start(
                out=xt0[0],
                in_=xT[0:512, 0:512].rearrange("(t p) n -> p t n", p=128))
            wks = cpool.tile([128, KT, 128], bf16, name="wks")
            nc.sync.dma_start(out=wks, in_=wk.rearrange("p (t m) -> p t m",
                                                        t=KT))
            nc.sync.dma_start(
                out=xt0[1],
                in_=xT[512:1024, 0:512].rearrange("(t p) n -> p t n", p=128))
            wvs = cpool.tile([128, KT, 128], bf16, name="wvs")
            nc.sync.dma_start(out=wvs, in_=wv.rearrange("p (t m) -> p t m",
                                                        t=KT))
            wos = cpool.tile([128, D], bf16, name="wos")
            nc.sync.dma_start(out=wos, in_=wo)

            # ---- small constants: one DMA on the gpsimd (SWDGE) queue,
            # masks generated on-device ----
            cst = cpool.tile([128, NCONST], bf16, name="cst")
            nc.gpsimd.dma_start(out=cst, in_=consts)
            NCV = CC_PER_B * 4 * HPC
            NVC = B * JT_PER_B * HPC
            cv = cst[:, 0:NCV].rearrange("p (c t h) -> p c t h",
                                         c=CC_PER_B, t=4)
            vcs = cst[:, NCV:NCV + NVC].rearrange("p (b t h) -> p b t h",
                                                  b=B, t=JT_PER_B)
            jb = cst[:, NCV + NVC:NCONST]
            ident = cpool.tile([128, 128], bf16, name="ident")
            make_identity(nc, ident)
            msk = cpool.tile([128, 128], bf16, name="msk")
            make_upper_triangular(nc, msk, 1.0, diag=True)

            # ---- persistent activations ----
            # qT/kT: [dh x 2 heads (h0 rows 0-63, h1 rows 64-127), B*N]
            qT = wpool.tile([128, NB], bf16, name="qT")
            kT = wpool.tile([128, NB], bf16, name="kT")
            # v natural + c_jt ones column: [j_loc, b, jtile, h, dh+1]
            vks = wpool.tile([128, B, JT_PER_B, HPC, DH + 1], bf16,
                             name="vks")
            nc.gpsimd.tensor_copy(
                out=vks[:, :, :, :, DH:DH + 1].rearrange(
                    "p b t h o -> p (b t h o)"),
                in_=vcs.rearrange("p b t h -> p (b t h)"))
            # normalized attention output, transposed: [dh x 2 heads, B*N]
            aT = wpool.tile([128, NB], bf16, name="aT")
            # softmax denominators' reciprocals
            rl = wpool.tile([128, HPC, 4], f32, name="rl")

            def load_chunk(g):
                if g == 0:
                    return xt0
                ts = []
                for hf in range(2):
                    xtc = xtpool.tile([128, KT // 2, 512], bf16, tag=f"x{hf}",
                                      bufs=3, name=f"xtc_{g}{hf}")
                    nc.sync.dma_start(
                        out=xtc,
                        in_=xT[512 * hf:512 * (hf + 1),
                               512 * g:512 * (g + 1)].rearrange(
                            "(t p) n -> p t n", p=128))
                    ts.append(xtc)
                return ts

            def proj_chunk(g, xtc):
                """rows [512g, 512g+512): project q/k/v from loaded chunk.
                Returns 3 thunks so q/k/v can be woven into the previous
                chunk's attention tail as PE bubble-filler."""
                b, cc = divmod(g, CC_PER_B)

                def proj_w(wsb, dst):
                    pp = pspool.tile([128, 512], f32, tag="pp", bufs=2,
                                     name=f"pp_{g}_{dst.tensor.name}")
                    for kt in range(KT):
                        nc.tensor.matmul(pp, wsb[:, kt, :],
                                         xtc[kt // 4][:, kt % 4, :],
                                         start=(kt == 0), stop=(kt == KT - 1))
                    nc.scalar.copy(dst[:, 512 * g:512 * (g + 1)], pp)

                def proj_v():
                    # v directly in natural layout: x-tile stationary.
                    # NOTE u-outer: accumulation groups sharing a PSUM bank
                    # must be sequential (interleaved groups corrupt each
                    # other). Slot 1 only ever reads j < 512 (its j-tile
                    # cap), so chunks with cc > 0 project slot 0's head only.
                    nh = HPC if cc == 0 else 1
                    w = DH * nh
                    vps = pspool.tile([128, 512], f32, tag="pp", bufs=2,
                                      name=f"vps_{g}")
                    for u in range(4):
                        for kt in range(KT):
                            nc.tensor.matmul(
                                vps[:, w * u:w * (u + 1)],
                                xtc[kt // 4][:, kt % 4,
                                             128 * u:128 * (u + 1)],
                                wvs[:, kt, 0:w],
                                start=(kt == 0), stop=(kt == KT - 1))
                    # scale by c_jt (and per-head layout) in one strided op
                    nc.vector.tensor_tensor(
                        out=vks[:, b, 4 * cc:4 * (cc + 1), 0:nh, 0:DH],
                        in0=vps[:, 0:4 * w].rearrange(
                            "p (t h d) -> p t h d", t=4, h=nh),
                        in1=cv[:, cc, :, 0:nh].rearrange(
                            "p t h -> p t h ()").broadcast_to(
                            (128, 4, nh, DH)),
                        op=ALU.mult)

                return [lambda: proj_w(wqs, qT), lambda: proj_w(wks, kT),
                        proj_v]

            def attention(b, cc, pending_ops, next_proj, finale=False):
                """q-chunk [512cc, 512cc+512) of batch b, both heads.
                next_proj: the next chunk's [q, k, v] projection thunks,
                woven into this chunk's attention tail as PE filler."""
                col = 2048 * b + 512 * cc
                njt = [min(4 * cc + 4, JT_CAPS[h]) for h in range(HPC)]
                npair = [n // 2 for n in njt]
                po2 = [pspool.tile([128, 4, DH + 1], f32, tag="po2", bufs=2,
                                   name=f"po2_{b}_{h}_{cc}")
                       for h in range(HPC)]

                def norm_head(h):
                    nc.vector.reciprocal(
                        rl[:, h, :],
                        po2[h][:, :, DH:DH + 1].rearrange("p u o -> p (u o)"))
                    nc.vector.tensor_tensor(
                        out=anat[:, :, DH * h:DH * (h + 1)],
                        in0=po2[h][:, :, 0:DH],
                        in1=rl[:, h, :].rearrange(
                            "p u -> p u ()").broadcast_to((128, 4, DH)),
                        op=ALU.mult)

                anat = spool.tile([128, 4, HPC * DH], bf16, tag="anat",
                                  name=f"anat_{b}_{cc}")

                # big per-head pt buffers: all of this chunk's exp'd score
                # tiles stay live so attn@v can run u-outer (sequential
                # accumulation groups per PSUM bank)
                ptc = [ptpool.tile([128, JT_CAPS[h], 512], bf16,
                                   tag=f"ptc{h}", name=f"ptc_{b}_{h}_{cc}")
                       for h in range(HPC)]

                def av_half(h, us):
                    for u in us:
                        stop_jt = min(4 * cc + u, njt[h] - 1)
                        for jt in range(stop_jt + 1):
                            nc.tensor.matmul(
                                po2[h][:, u, :],
                                ptc[h][:, jt, 128 * u:128 * (u + 1)],
                                vks[:, b, jt, h, :],
                                start=(jt == 0), stop=(jt == stop_jt))

                def av_head(h):
                    av_half(h, (0, 1, 2, 3))

                for pr in range(max(npair)):
                    if pr >= 1 and pending_ops:
                        pending_ops.pop(0)()
                    if pr == npair[1] and npair[1] < npair[0]:
                        # capped slot finished a pair ago (its exp has had a
                        # full pair of slack): its attn@v + norm are ready
                        # PE/DVE filler, and free its PSUM slot early
                        av_head(1)
                        norm_head(1)
                    ptl = {}
                    for h in range(HPC):
                        if pr >= npair[h]:
                            continue
                        ps = pspool.tile([128, 2, 512], f32, tag="big",
                                         bufs=2, name=f"ps_{b}_{h}_{cc}_{pr}")
                        for m in range(2):
                            jt = 2 * pr + m
                            j0 = 2048 * b + 128 * jt
                            c0 = max(0, 128 * (jt - 4 * cc))
                            nc.tensor.matmul(
                                ps[:, m, c0:512],
                                kT[64 * h:64 * (h + 1), j0:j0 + 128],
                                qT[64 * h:64 * (h + 1),
                                   col + c0:col + 512],
                                start=True, stop=True)
                        ptl[h] = ps
                    c0p = max(0, 128 * (2 * pr - 4 * cc))
                    for h in range(HPC):
                        if pr >= npair[h]:
                            continue
                        pt = ptc[h][:, 2 * pr:2 * pr + 2, :]
                        nc.scalar.activation(pt[:, :, c0p:512],
                                             ptl[h][:, :, c0p:512], AF.Exp,
                                             bias=jb[:, h:h + 1], scale=1.0)
                        for m in range(2):
                            o4 = 2 * pr + m - 4 * cc
                            if o4 >= 0:
                                # diagonal tile: zero the triangle
                                nc.vector.tensor_tensor(
                                    out=pt[:, m, 128 * o4:128 * (o4 + 1)],
                                    in0=pt[:, m, 128 * o4:128 * (o4 + 1)],
                                    in1=msk, op=ALU.mult)

                if npair[1] >= npair[0]:
                    av_head(1)
                    norm_head(1)

                # u0/u1 only need j-tiles through the second-to-last pair;
                # weave the next chunk's projections around the final exp
                av_half(0, (0, 1))
                if next_proj:
                    next_proj[0]()
                    next_proj[1]()
                av_half(0, (2, 3))
                if next_proj:
                    for op in next_proj[2:]:
                        op()
                for op in pending_ops:
                    op()
                del pending_ops[:]

                def norm(eng=None):
                    norm_head(0)
                    # transpose A_nat -> aT per q-tile; the 4 transposed
                    # tiles land contiguously so one copy moves them all
                    tps = pspool.tile([128, 512], f32, tag="pp", bufs=2,
                                      name=f"tps_{b}_{cc}")
                    tpb = tps.bitcast(bf16)  # [128, 1024] bf16 view
                    for u in range(4):
                        nc.tensor.transpose(tpb[:, 128 * u:128 * (u + 1)],
                                            anat[:, u, :], ident)
                    nc.vector.tensor_copy(out=aT[:, col:col + 512],
                                          in_=tpb[:, 0:512])
                return norm

            def wo_ops(b, cc, last=False, npops=0):
                """Per-qtile-half Wo emitters; interleaved into the next
                chunk's attention loop as PE bubble-filler (or emitted
                immediately, finely chunked, for the last chunk). The first
                `npops` entries are single-half ops (popped mid-pair, PSUM
                "pp" tag); the rest run in the consumer's tail where the
                scores banks are free, as 2-halves-per-op bulk entries."""
                ops = []
                half_budget = npops
                for qp in range(8 * b + 2 * cc, 8 * b + 2 * (cc + 1)):
                    osb = opool.tile([128, 2, D], bf16, tag="osb",
                                     name=f"osb_{qp}")
                    for u in range(2):
                        qt = 2 * qp + u
                        if not last and half_budget < 2:
                            def opb(eng=None, qt=qt, u=u, osb=osb):
                                pwb = pspool.tile([128, 2, 512], f32,
                                                  tag="big", bufs=2,
                                                  name=f"pwb_{qt}")
                                for half in range(2):
                                    nc.tensor.matmul(
                                        pwb[:, half, :],
                                        aT[:, 128 * qt:128 * (qt + 1)],
                                        wos[:, 512 * half:512 * (half + 1)],
                                        start=True, stop=True)
                                nc.vector.tensor_copy(
                                    out=osb[:, u, :].rearrange(
                                        "p (m d) -> p m d", m=2),
                                    in_=pwb)
                                if u == 1:
                                    nc.gpsimd.dma_start(
                                        out=out[256 * (qt // 2):
                                                256 * (qt // 2 + 1),
                                                :].rearrange(
                                            "(t p) d -> p t d", p=128),
                                        in_=osb)
                            ops.append(opb)
                            continue
                        half_budget -= 2
                        if last:
                            # tail path: both halves in one 2-bank PSUM tile
                            # (scores' banks are free by now), one copy, and
                            # low-latency per-qtile stores on the idle sync
                            # queue
                            def opl(qt=qt, u=u, osb=osb):
                                pwb = pspool.tile([128, 2, 512], f32,
                                                  tag="big", bufs=2,
                                                  name=f"pwb_{qt}")
                                for half in range(2):
                                    nc.tensor.matmul(
                                        pwb[:, half, :],
                                        aT[:, 128 * qt:128 * (qt + 1)],
                                        wos[:, 512 * half:512 * (half + 1)],
                                        start=True, stop=True)
                                nc.vector.tensor_copy(
                                    out=osb[:, u, :].rearrange(
                                        "p (m d) -> p m d", m=2),
                                    in_=pwb)
                                nc.sync.dma_start(
                                    out=out[128 * qt:128 * (qt + 1),
                                            :].rearrange(
                                        "(t p) d -> p t d", p=128),
                                    in_=osb[:, u:u + 1, :])
                            ops.append(opl)
                            continue
                        for half in range(2):
                            def op(eng=None, qp=qp, u=u, qt=qt, half=half,
                                   osb=osb):
                                pw = pspool.tile([128, 512], f32, tag="pp",
                                                 bufs=2,
                                                 name=f"pw_{qt}_{half}")
                                nc.tensor.matmul(
                                    pw,
                                    aT[:, 128 * qt:128 * (qt + 1)],
                                    wos[:, 512 * half:512 * (half + 1)],
                                    start=True, stop=True)
                                dst = osb[:, u, 512 * half:512 * (half + 1)]
                                if eng is nc.scalar:
                                    nc.scalar.copy(dst, pw)
                                else:
                                    nc.vector.tensor_copy(out=dst, in_=pw)
                                if u == 1 and half == 1:
                                    nc.gpsimd.dma_start(
                                        out=out[256 * qp:256 * (qp + 1),
                                                :].rearrange(
                                            "(t p) d -> p t d", p=128),
                                        in_=osb)
                            ops.append(op)
                return ops

            for rep in range(repeat):
                pending = []
                cur = load_chunk(0)
                for op in proj_chunk(0, cur):
                    op()
                nxt = load_chunk(1)
                for b in range(B):
                    for cc in range(CC_PER_B):
                        g = CC_PER_B * b + cc
                        # prefetch 2 chunks ahead so loads sit in front of
                        # stores on the (serialized) DMA engines
                        nxt2 = load_chunk(g + 2) if g + 2 < NCHUNK else None
                        next_proj = (proj_chunk(g + 1, nxt)
                                     if g + 1 < NCHUNK else None)
                        norm_fn = attention(b, cc, pending, next_proj)
                        if g == NCHUNK - 1:
                            norm_fn()
                            for op in wo_ops(b, cc, last=True):
                                op()
                        else:
                            # norm (incl. transposes) is deferred into the
                            # next chunk's attention as its first pending op.
                            # The consumer pops npair'-1 ops mid-loop; size
                            # the single-half op count to that budget.
                            cc2 = (g + 1) % CC_PER_B
                            npops = min(4 * cc2 + 4, JT_CAPS[0]) // 2 - 2
                            pending = [norm_fn] + wo_ops(b, cc, npops=npops)
                        nxt = nxt2

    nc.finalize()
    return nc


_CACHE = {}


def _get_program():
    if "nc" not in _CACHE:
        _CACHE["nc"] = build_program()
    return _CACHE["nc"]


def _make_in_maps(x, Wq, Wk, Wv, Wo):
    import ml_dtypes
    b16 = ml_dtypes.bfloat16

    def tile_w(w):
        # [D, 128] -> [128, KT*128] partition-major tiling
        return np.ascontiguousarray(
            w.reshape(KT, 128, HPC * DH).transpose(1, 0, 2).reshape(
                128, KT * 128).astype(b16))

    x2 = np.ascontiguousarray(
        x.reshape(NB, D).astype(np.float32).T).astype(b16)
    base = (2.0 ** 8) ** (1.0 / H)
    slopes = 1.0 / base ** np.arange(1, H + 1, dtype=np.float64)
    jl = np.arange(128)
    scale = DH ** -0.5
    in_maps = []
    with np.errstate(under="ignore"):
        for c in range(NCORES):
            heads = [15 - c, c]
            cols = np.concatenate([np.arange(64 * h, 64 * (h + 1))
                                   for h in heads])
            sl = slopes[heads]                      # [HPC]
            # per-partition exp bias: -slope_h * j_loc
            jbv = (-sl[None, :] * jl[:, None]).astype(b16)  # [128, HPC]
            # c_jt = exp(-128*slope*jt), folded onto V blocks
            cjt = np.exp(-128.0 * sl[None, :] *
                         np.arange(JT_PER_B, dtype=np.float64)[:, None])
            # cv[(cc t h)] = c(4cc+t, h);  vcol[(b jt h)] = c(jt, h)
            cvv = np.broadcast_to(
                cjt.astype(b16).reshape(1, CC_PER_B * 4 * HPC),
                (128, CC_PER_B * 4 * HPC))
            vc = np.broadcast_to(
                np.tile(cjt.astype(b16).reshape(1, JT_PER_B * HPC),
                        (1, B)),
                (128, B * JT_PER_B * HPC))
            cpk = np.concatenate([cvv, vc, jbv], axis=1)
            in_maps.append({
                "xT": x2,
                "wq": tile_w(Wq[:, cols].astype(np.float32) * scale),
                "wk": tile_w(Wk[:, cols].astype(np.float32)),
                "wv": tile_w(Wv[:, cols].astype(np.float32)),
                "wo": np.ascontiguousarray(Wo[cols, :]).astype(b16),
                "consts": np.ascontiguousarray(cpk),
            })
    return in_maps


def run_cores(x, Wq, Wk, Wv, Wo, **spmd_kwargs):
    nc = _get_program()
    in_maps = _make_in_maps(x, Wq, Wk, Wv, Wo)
    return run_bass_kernel_spmd(nc, in_maps, list(range(NCORES)),
                                **spmd_kwargs)


def kernel(x, Wq, Wk, Wv, Wo, bo):
    res = run_cores(np.asarray(x), np.asarray(Wq), np.asarray(Wk),
                    np.asarray(Wv), np.asarray(Wo))
    acc = np.zeros((NB, D), dtype=np.float64)
    for r in res.results:
        acc += np.asarray(r["out"], dtype=np.float64)
    acc += np.asarray(bo, dtype=np.float64)[None, :]
    return acc.astype(np.float32).reshape(B, N, D)
